# revision 35
# baseline (speedup 1.0000x reference)
"""Multi-head causal self-attention (B=2, T=2048, D=1024, H=16) on 8 trn2 cores.

Sharding: data-parallel over batch (cores 0-3 -> batch 0, 4-7 -> batch 1),
tensor-parallel over heads within each 4-core group (4 heads per core).
Wq/Wk/Wv column-sharded, Wo row-sharded; each core emits its partial output
projection and the host sums the 4 partials per batch (TP unshard).

Per-core pipeline (bf16 operands, fp32 PSUM):
  qT/kT = W_slice @ x^T; v = x @ Wv^T (+ones column for softmax denominator)
  per (512-query block qb, head-pair hp), per 128-key tile kt:
     scores (row-packed K=64 matmul pairs, N restricted to non-masked queries)
     e = exp(0.125*s) -> bf16 SBUF (restricted)
     causal mask on diagonal tiles: DVE multiply with a triangle mask
     oX[65, 512] += v_aug.T @ e  (rows 0-63 att out, row 64 denominator)
  normalize: DVE reciprocal_approx_fast + gpsimd partition_broadcast + DVE mul
  oproj per token tile, interleaved as PE filler into the next block's
  attention loop. Projection matmuls of block qb+1 are likewise interleaved
  into block qb's attention so exp (ACT engine) overlaps PE work.
"""

import sys
from collections import deque

for _p in ("/opt/trn_rl_repo", "/root/.axon_site/_ro/trn_rl_repo"):
    if _p not in sys.path:
        sys.path.append(_p)

import numpy as np

import concourse.bass as bass
import concourse.mybir as mybir
import concourse.tile as tile
from concourse import bacc
from concourse.bass_utils import run_bass_kernel_spmd

F32 = mybir.dt.float32
BF16 = mybir.dt.bfloat16

B, T, D = 2, 2048, 1024
H, DH = 16, 64
HPC = 4          # heads per core
FPC = HPC * DH   # feature dims per core (256)
NKT = T // 128   # 16 key tiles / token tiles
NQB = T // 512   # 4 query blocks
VW = DH + 1      # v width incl ones column (65)

DEBUG_DUMPS = False  # add qT/kT/v/attT DRAM dumps for HW debugging

_CACHE = {}


def _build():
    nc = bacc.Bacc("TRN2", target_bir_lowering=False, debug=False, num_devices=8)

    xt_d = nc.dram_tensor("xtp", [128, 8 * T], BF16, kind="ExternalInput").ap()
    wq_d = nc.dram_tensor("wq_t", [128, 8 * FPC], BF16, kind="ExternalInput").ap()
    wk_d = nc.dram_tensor("wk_t", [128, 8 * FPC], BF16, kind="ExternalInput").ap()
    wv_d = nc.dram_tensor("wv_t", [128, 8 * FPC], BF16, kind="ExternalInput").ap()
    wo_d = nc.dram_tensor("wo_t", [128, 2 * D], BF16, kind="ExternalInput").ap()
    msk_d = nc.dram_tensor("msk", [128, 256], BF16, kind="ExternalInput").ap()
    out_d = nc.dram_tensor("po", [T, D], BF16, kind="ExternalOutput").ap()

    with tile.TileContext(nc) as tc:
        with (
            tc.tile_pool(name="wp", bufs=1) as wp,
            tc.tile_pool(name="ep", bufs=4) as ep,
            tc.tile_pool(name="nr", bufs=2) as nr,
            tc.tile_pool(name="op", bufs=3) as op,
            tc.tile_pool(name="ps", bufs=1, space="PSUM") as ps,
        ):
            # ---- persistent SBUF ----
            # DMA priority: weights for tb0's units first, then tb0's x
            # columns, so the first matmul can start ~5us in instead of ~19us.
            wq_sb = wp.tile([128, 8 * FPC], BF16, tag="wq")
            nc.sync.dma_start(wq_sb[:], wq_d)
            wk_sb = wp.tile([128, 8 * FPC], BF16, tag="wk")
            nc.sync.dma_start(wk_sb[:], wk_d)
            # x host-prepacked chunk-major [128, (kc, t)]: one strided DMA per
            # token block (DMA *instruction issue* on the sync sequencer is
            # the startup bottleneck, so fewer, larger DMAs win)
            x_sb = wp.tile([128, 8 * T], BF16, tag="xp")
            xr = x_sb[:].rearrange("p (c t) -> p c t", t=T)
            xsr = xt_d.rearrange("p (c t) -> p c t", t=T)

            def dma_x_cols(tb):
                nc.sync.dma_start(
                    xr[:, :, tb * 512 : (tb + 1) * 512],
                    xsr[:, :, tb * 512 : (tb + 1) * 512],
                )

            dma_x_cols(0)
            wv_sb = wp.tile([128, 8 * FPC], BF16, tag="wv")
            nc.sync.dma_start(wv_sb[:], wv_d)

            qT_sb = wp.tile([128, 2 * T], BF16, tag="qT")  # head-pair hp at cols hp*T
            kT_sb = wp.tile([128, 2 * T], BF16, tag="kT")
            v_sb = wp.tile([128, NKT * HPC * VW], BF16, tag="v")
            vr = v_sb[:].rearrange("p (t h d) -> p t h d", h=HPC, d=VW)
            # ones column (softmax denominator row of the attv psum).
            # memset, NOT a DMA: a strided DMA into v_sb races with the
            # v-unit copies at burst granularity.
            nc.gpsimd.memset(vr[:, :, :, 64], 1.0)
            msk_sb = wp.tile([128, 256], BF16, tag="msk")
            nc.sync.dma_start(msk_sb[:], msk_d)
            mskr = msk_sb[:].rearrange("p (h q) -> p h q", h=2)
            wo_sb = wp.tile([128, 2 * D], BF16, tag="wo")
            nc.sync.dma_start(wo_sb[:], wo_d)
            for tb in range(1, NQB):
                dma_x_cols(tb)
            attT_sb = wp.tile([128, 2 * T], BF16, tag="attT")

            pslot = [0]  # round-robin proj psum slot

            def qk_unit(tb, hp, w_sb, dst):
                p_ps = ps.tile([128, 512], F32, tag=f"p{pslot[0]}", bufs=1)
                pslot[0] ^= 1
                for kc in range(8):
                    nc.tensor.matmul(
                        p_ps[:],
                        w_sb[:, kc * FPC + hp * 128 : kc * FPC + (hp + 1) * 128],
                        xr[:, kc, tb * 512 : (tb + 1) * 512],
                        start=(kc == 0), stop=(kc == 7),
                    )
                nc.vector.tensor_copy(
                    dst[:, hp * T + tb * 512 : hp * T + (tb + 1) * 512], p_ps[:]
                )

            def v_unit(tp):  # token-tile pair tp -> tiles 2tp, 2tp+1
                p_ps = ps.tile([128, 512], F32, tag=f"p{pslot[0]}", bufs=1)
                pslot[0] ^= 1
                for j in (0, 1):
                    tt = 2 * tp + j
                    for kc in range(8):
                        nc.tensor.matmul(
                            p_ps[:, j * 256 : (j + 1) * 256],
                            xr[:, kc, tt * 128 : (tt + 1) * 128],
                            wv_sb[:, kc * FPC : (kc + 1) * FPC],
                            start=(j == 0 and kc == 0), stop=(j == 1 and kc == 7),
                        )
                for j in (0, 1):
                    nc.vector.tensor_copy(
                        vr[:, 2 * tp + j, :, 0:DH],
                        p_ps[:, j * 256 : (j + 1) * 256].rearrange(
                            "p (h d) -> p h d", d=DH
                        ),
                    )

            def wo_unit(tt, odh, o_sb, act_copy=False):
                wo_ps = ps.tile([128, 512], F32, tag=f"p{pslot[0]}", bufs=1)
                pslot[0] ^= 1
                for hp in (0, 1):
                    nc.tensor.matmul(
                        wo_ps[:],
                        attT_sb[:, hp * T + tt * 128 : hp * T + (tt + 1) * 128],
                        wo_sb[:, hp * D + odh * 512 : hp * D + (odh + 1) * 512],
                        start=(hp == 0), stop=(hp == 1),
                    )
                dst = o_sb[:, odh * 512 : (odh + 1) * 512]
                if act_copy:  # drain tail: DVE is busy with normalize, ACT idle
                    nc.scalar.copy(dst, wo_ps[:])
                else:
                    nc.vector.tensor_copy(dst, wo_ps[:])
                if odh == 1:
                    nc.sync.dma_start(out_d[tt * 128 : (tt + 1) * 128, :], o_sb[:])

            # filler queue: (need_key, fn). need_key = (qb, hp) by which the
            # unit MUST have run (data dependency); (99, 0) = no deadline.
            fillers = deque()

            def proj_units(tb):
                yield (tb, 0), lambda: qk_unit(tb, 0, wq_sb, qT_sb)
                yield (tb, 0), lambda: qk_unit(tb, 0, wk_sb, kT_sb)
                yield (tb, 0), lambda: v_unit(2 * tb)
                yield (tb, 0), lambda: v_unit(2 * tb + 1)
                yield (tb, 1), lambda: qk_unit(tb, 1, wq_sb, qT_sb)
                yield (tb, 1), lambda: qk_unit(tb, 1, wk_sb, kT_sb)

            def oproj_units(qb, act_copy=False):
                for t4 in range(4):
                    tt = 4 * qb + t4
                    o_sb = op.tile([128, D], BF16, tag="osb")
                    for odh in (0, 1):
                        yield (99, 0), (
                            lambda tt=tt, odh=odh, o_sb=o_sb: wo_unit(
                                tt, odh, o_sb, act_copy
                            )
                        )

            def drain(n):
                for _ in range(min(n, len(fillers))):
                    fillers.popleft()[1]()

            def force_drain(key):
                while fillers and fillers[0][0] <= key:
                    fillers.popleft()[1]()

            def attn_block(qb, hp):
                force_drain((qb, hp))  # units this block's data depends on
                nkt = 4 * (qb + 1)
                oA = ps.tile([VW, 512], F32, tag="oA", bufs=1)
                oB = ps.tile([VW, 512], F32, tag="oB", bufs=1)
                # spread fillers over BOTH hp blocks of this qb
                budget = len(fillers)
                per_kt = -(-budget // ((2 - hp) * nkt)) if budget else 0
                for kt in range(nkt):
                    r = kt - 4 * qb
                    qlo = 128 * r if r > 0 else 0
                    s_t = ps.tile([128, 1024], F32, tag="s", bufs=2)
                    for h in (0, 1):
                        nc.tensor.matmul(
                            s_t[:, h * 512 + qlo : (h + 1) * 512],
                            kT_sb[h * 64 : (h + 1) * 64,
                                  hp * T + kt * 128 : hp * T + (kt + 1) * 128],
                            qT_sb[h * 64 : (h + 1) * 64,
                                  hp * T + qb * 512 + qlo : hp * T + (qb + 1) * 512],
                            start=True, stop=True, tile_position=(h * 64, 0),
                        )
                    e_t = ep.tile([128, 1024], BF16, tag="e")
                    if r > 0:
                        src = s_t[:].rearrange("p (h q) -> p h q", h=2)[:, :, qlo:]
                        dst = e_t[:].rearrange("p (h q) -> p h q", h=2)[:, :, qlo:]
                    else:
                        src = s_t[:]
                        dst = e_t[:]
                    nc.scalar.activation(
                        dst, src, mybir.ActivationFunctionType.Exp, scale=0.125
                    )
                    if r >= 0:  # diagonal: mask q < k + 128r
                        # (cols < 128r are skipped by the attv N-restriction,
                        # so only the triangle zone needs masking). DVE mask
                        # multiply, NOT gpsimd affine_select: mixing gpsimd op
                        # types forces Q7 library reloads that stall the
                        # partition_broadcast on the block tails.
                        er = e_t[:].rearrange("p (h q) -> p h q", h=2)
                        ape = er[:, :, 128 * r : 128 * (r + 1)]
                        nc.vector.tensor_mul(ape, ape, mskr)
                    drain(per_kt)
                    for h, oX in ((0, oA), (1, oB)):
                        nc.tensor.matmul(
                            oX[:, qlo:512],
                            vr[:, kt, 2 * hp + h, :],
                            e_t[:, h * 512 + qlo : (h + 1) * 512],
                            start=(kt == 0), stop=(kt == nkt - 1),
                        )
                # normalize -> attT. Stage oX into SBUF with one copy per head
                # (frees the PSUM bank for the next block immediately), then
                # bounce the denominator row to a base-0 tile: reciprocal_
                # approx_fast gives garbage on PSUM or non-base-0 operands.
                os_sb = nr.tile([VW, 1024], F32, tag="os")
                nc.vector.tensor_copy(os_sb[:, 0:512], oA[:])
                nc.vector.tensor_copy(os_sb[:, 512:1024], oB[:])
                dn = nr.tile([1, 1024], F32, tag="dn")
                nc.vector.tensor_copy(dn[:], os_sb[64:65, :])
                rr = nr.tile([1, 1024], F32, tag="rr")
                nc.vector.reciprocal_approx_fast(rr[:], dn[:])
                bc = nr.tile([64, 1024], F32, tag="bc")
                nc.gpsimd.partition_broadcast(bc[:], rr[:])
                cols = slice(hp * T + qb * 512, hp * T + (qb + 1) * 512)
                nc.vector.tensor_mul(
                    attT_sb[0:64, cols], os_sb[0:64, 0:512], bc[:, 0:512]
                )
                nc.vector.tensor_mul(
                    attT_sb[64:128, cols], os_sb[0:64, 512:1024], bc[:, 512:1024]
                )
                if DEBUG_DUMPS:
                    for nm, t in ((f"dos{qb}{hp}", dn), (f"drr{qb}{hp}", rr),
                                  (f"dbc{qb}{hp}", bc)):
                        dd = nc.dram_tensor(
                            nm, list(t.shape), F32, kind="ExternalOutput"
                        )
                        nc.sync.dma_start(dd.ap(), t[:])

            # ---- main schedule ----
            for _, u in proj_units(0):
                u()
            fillers.extend(proj_units(1))
            fillers.extend(proj_units(2))
            for qb in range(NQB):
                if qb == 1:
                    fillers.extend(proj_units(3))
                if qb > 0:
                    fillers.extend(oproj_units(qb - 1))
                attn_block(qb, 0)
                attn_block(qb, 1)
            drain(len(fillers))
            for _, u in oproj_units(NQB - 1, act_copy=True):
                u()

            if DEBUG_DUMPS:
                for nm, t in (
                    ("dq", qT_sb), ("dk", kT_sb), ("dv", v_sb), ("datt", attT_sb)
                ):
                    d = nc.dram_tensor(nm, list(t.shape), BF16, kind="ExternalOutput")
                    nc.sync.dma_start(d.ap(), t[:])

    nc.compile()
    return nc


def _prepack(w, dt):
    # [c*128, f] -> [128, c*f] (SBUF chunk layout)
    c = w.shape[0] // 128
    return np.ascontiguousarray(
        w.reshape(c, 128, w.shape[1]).transpose(1, 0, 2).reshape(128, -1)
    ).astype(dt)


def _prep_in_maps(x, Wq, Wk, Wv, Wo):
    x = np.asarray(x, dtype=np.float32)
    bf = mybir.dt.np(BF16)
    Wq = np.asarray(Wq, dtype=np.float32)
    Wk = np.asarray(Wk, dtype=np.float32)
    Wv = np.asarray(Wv, dtype=np.float32)
    Wo = np.asarray(Wo, dtype=np.float32)
    ii = np.arange(128)[:, None]
    jj = np.arange(128)[None, :]
    msk = np.tile((jj >= ii).astype(bf), (1, 2))
    in_maps = []
    for c in range(8):
        b, g = divmod(c, 4)
        sl = slice(g * FPC, (g + 1) * FPC)
        in_maps.append(
            {
                "xtp": _prepack(np.ascontiguousarray(x[b].T), bf),
                "wq_t": _prepack(Wq[sl, :].T, bf),
                "wk_t": _prepack(Wk[sl, :].T, bf),
                "wv_t": _prepack(Wv[sl, :].T, bf),
                "wo_t": _prepack(Wo[:, sl].T, bf),
                "msk": msk,
            }
        )
    return in_maps


def _get_nc():
    if "nc" not in _CACHE:
        _CACHE["nc"] = _build()
    return _CACHE["nc"]


def _assemble(results):
    out = np.empty((B, T, D), dtype=np.float32)
    for b in range(B):
        out[b] = (
            results[4 * b]["po"].astype(np.float32)
            + results[4 * b + 1]["po"].astype(np.float32)
            + results[4 * b + 2]["po"].astype(np.float32)
            + results[4 * b + 3]["po"].astype(np.float32)
        )
    return out


def kernel(x, Wq, Wk, Wv, Wo):
    nc = _get_nc()
    in_maps = _prep_in_maps(x, Wq, Wk, Wv, Wo)
    res = run_bass_kernel_spmd(nc, in_maps, core_ids=list(range(8)))
    return _assemble(res.results)


def kernel_with_trace(x, Wq, Wk, Wv, Wo, **kw):
    nc = _get_nc()
    in_maps = _prep_in_maps(x, Wq, Wk, Wv, Wo)
    res = run_bass_kernel_spmd(nc, in_maps, core_ids=list(range(8)), trace=True, **kw)
    return _assemble(res.results), res


# revision 37
# speedup vs baseline: 1.0237x; 1.0237x over previous
"""Multi-head causal self-attention (B=2, T=2048, D=1024, H=16) on 8 trn2 cores.

Sharding: data-parallel over batch (cores 0-3 -> batch 0, 4-7 -> batch 1),
tensor-parallel over heads within each 4-core group (4 heads per core).
Wq/Wk/Wv column-sharded, Wo row-sharded; each core emits its partial output
projection and the host sums the 4 partials per batch (TP unshard).

Per-core pipeline (bf16 operands, fp32 PSUM):
  qT/kT = W_slice @ x^T; v = x @ Wv^T (+ones column for softmax denominator)
  per (512-query block qb, head-pair hp), per 128-key tile kt:
     scores (row-packed K=64 matmul pairs, N restricted to non-masked queries)
     e = exp(0.125*s) -> bf16 SBUF (restricted)
     causal mask on diagonal tiles: DVE multiply with a triangle mask
     oX[65, 512] += v_aug.T @ e  (rows 0-63 att out, row 64 denominator)
  normalize: DVE reciprocal_approx_fast + gpsimd partition_broadcast + DVE mul
  oproj per token tile, interleaved as PE filler into the next block's
  attention loop. Projection matmuls of block qb+1 are likewise interleaved
  into block qb's attention so exp (ACT engine) overlaps PE work.
"""

import sys
from collections import deque

for _p in ("/opt/trn_rl_repo", "/root/.axon_site/_ro/trn_rl_repo"):
    if _p not in sys.path:
        sys.path.append(_p)

import numpy as np

import concourse.bass as bass
import concourse.mybir as mybir
import concourse.tile as tile
from concourse import bacc
from concourse.bass_utils import run_bass_kernel_spmd

F32 = mybir.dt.float32
BF16 = mybir.dt.bfloat16

B, T, D = 2, 2048, 1024
H, DH = 16, 64
HPC = 4          # heads per core
FPC = HPC * DH   # feature dims per core (256)
NKT = T // 128   # 16 key tiles / token tiles
NQB = T // 512   # 4 query blocks
VW = DH + 1      # v width incl ones column (65)

DEBUG_DUMPS = False  # add qT/kT/v/attT DRAM dumps for HW debugging

_CACHE = {}


def _build():
    nc = bacc.Bacc("TRN2", target_bir_lowering=False, debug=False, num_devices=8)

    xt_d = nc.dram_tensor("xt", [D, T], BF16, kind="ExternalInput").ap()
    wq_d = nc.dram_tensor("wq_t", [128, 8 * FPC], BF16, kind="ExternalInput").ap()
    wk_d = nc.dram_tensor("wk_t", [128, 8 * FPC], BF16, kind="ExternalInput").ap()
    wv_d = nc.dram_tensor("wv_t", [128, 8 * FPC], BF16, kind="ExternalInput").ap()
    wo_d = nc.dram_tensor("wo_t", [128, 2 * D], BF16, kind="ExternalInput").ap()
    msk_d = nc.dram_tensor("msk", [128, 256], BF16, kind="ExternalInput").ap()
    out_d = nc.dram_tensor("po", [T, D], BF16, kind="ExternalOutput").ap()

    with tile.TileContext(nc) as tc:
        with (
            tc.tile_pool(name="wp", bufs=1) as wp,
            tc.tile_pool(name="ep", bufs=4) as ep,
            tc.tile_pool(name="nr", bufs=2) as nr,
            tc.tile_pool(name="op", bufs=3) as op,
            tc.tile_pool(name="ps", bufs=1, space="PSUM") as ps,
        ):
            # ---- persistent SBUF ----
            # DMA priority: weights for tb0's units first, then tb0's x
            # columns, so the first matmul can start ~5us in instead of ~19us.
            wq_sb = wp.tile([128, 8 * FPC], BF16, tag="wq")
            nc.sync.dma_start(wq_sb[:], wq_d)
            wk_sb = wp.tile([128, 8 * FPC], BF16, tag="wk")
            nc.sync.dma_start(wk_sb[:], wk_d)
            xT = [
                wp.tile([128, T], BF16, tag=f"xT{kc}", name=f"xT{kc}")
                for kc in range(8)
            ]

            def dma_x_cols(tb):
                for kc in range(8):
                    nc.sync.dma_start(
                        xT[kc][:, tb * 512 : (tb + 1) * 512],
                        xt_d[kc * 128 : (kc + 1) * 128, tb * 512 : (tb + 1) * 512],
                    )

            dma_x_cols(0)
            wv_sb = wp.tile([128, 8 * FPC], BF16, tag="wv")
            nc.sync.dma_start(wv_sb[:], wv_d)

            qT_sb = wp.tile([128, 2 * T], BF16, tag="qT")  # head-pair hp at cols hp*T
            kT_sb = wp.tile([128, 2 * T], BF16, tag="kT")
            v_sb = wp.tile([128, NKT * HPC * VW], BF16, tag="v")
            vr = v_sb[:].rearrange("p (t h d) -> p t h d", h=HPC, d=VW)
            # ones column (softmax denominator row of the attv psum).
            # memset, NOT a DMA: a strided DMA into v_sb races with the
            # v-unit copies at burst granularity.
            nc.gpsimd.memset(vr[:, :, :, 64], 1.0)
            msk_sb = wp.tile([128, 256], BF16, tag="msk")
            nc.sync.dma_start(msk_sb[:], msk_d)
            mskr = msk_sb[:].rearrange("p (h q) -> p h q", h=2)
            wo_sb = wp.tile([128, 2 * D], BF16, tag="wo")
            nc.sync.dma_start(wo_sb[:], wo_d)
            for tb in range(1, NQB):
                dma_x_cols(tb)
            attT_sb = wp.tile([128, 2 * T], BF16, tag="attT")

            pslot = [0]  # round-robin proj psum slot

            def qk_unit(tb, hp, w_sb, dst):
                p_ps = ps.tile([128, 512], F32, tag=f"p{pslot[0]}", bufs=1)
                pslot[0] ^= 1
                for kc in range(8):
                    nc.tensor.matmul(
                        p_ps[:],
                        w_sb[:, kc * FPC + hp * 128 : kc * FPC + (hp + 1) * 128],
                        xT[kc][:, tb * 512 : (tb + 1) * 512],
                        start=(kc == 0), stop=(kc == 7),
                    )
                nc.vector.tensor_copy(
                    dst[:, hp * T + tb * 512 : hp * T + (tb + 1) * 512], p_ps[:]
                )

            def v_unit(tp):  # token-tile pair tp -> tiles 2tp, 2tp+1
                p_ps = ps.tile([128, 512], F32, tag=f"p{pslot[0]}", bufs=1)
                pslot[0] ^= 1
                for j in (0, 1):
                    tt = 2 * tp + j
                    for kc in range(8):
                        nc.tensor.matmul(
                            p_ps[:, j * 256 : (j + 1) * 256],
                            xT[kc][:, tt * 128 : (tt + 1) * 128],
                            wv_sb[:, kc * FPC : (kc + 1) * FPC],
                            start=(j == 0 and kc == 0), stop=(j == 1 and kc == 7),
                        )
                for j in (0, 1):
                    nc.vector.tensor_copy(
                        vr[:, 2 * tp + j, :, 0:DH],
                        p_ps[:, j * 256 : (j + 1) * 256].rearrange(
                            "p (h d) -> p h d", d=DH
                        ),
                    )

            def wo_unit(tt, odh, o_sb, act_copy=False):
                wo_ps = ps.tile([128, 512], F32, tag=f"p{pslot[0]}", bufs=1)
                pslot[0] ^= 1
                for hp in (0, 1):
                    nc.tensor.matmul(
                        wo_ps[:],
                        attT_sb[:, hp * T + tt * 128 : hp * T + (tt + 1) * 128],
                        wo_sb[:, hp * D + odh * 512 : hp * D + (odh + 1) * 512],
                        start=(hp == 0), stop=(hp == 1),
                    )
                dst = o_sb[:, odh * 512 : (odh + 1) * 512]
                if act_copy:  # drain tail: DVE is busy with normalize, ACT idle
                    nc.scalar.copy(dst, wo_ps[:])
                else:
                    nc.vector.tensor_copy(dst, wo_ps[:])
                if odh == 1:
                    nc.sync.dma_start(out_d[tt * 128 : (tt + 1) * 128, :], o_sb[:])

            # filler queue: (need_key, fn). need_key = (qb, hp) by which the
            # unit MUST have run (data dependency); (99, 0) = no deadline.
            fillers = deque()

            def proj_units(tb):
                yield (tb, 0), lambda: qk_unit(tb, 0, wq_sb, qT_sb)
                yield (tb, 0), lambda: qk_unit(tb, 0, wk_sb, kT_sb)
                yield (tb, 0), lambda: v_unit(2 * tb)
                yield (tb, 0), lambda: v_unit(2 * tb + 1)
                yield (tb, 1), lambda: qk_unit(tb, 1, wq_sb, qT_sb)
                yield (tb, 1), lambda: qk_unit(tb, 1, wk_sb, kT_sb)

            def oproj_units(qb, act_copy=False):
                for t4 in range(4):
                    tt = 4 * qb + t4
                    o_sb = op.tile([128, D], BF16, tag="osb")
                    for odh in (0, 1):
                        yield (99, 0), (
                            lambda tt=tt, odh=odh, o_sb=o_sb: wo_unit(
                                tt, odh, o_sb, act_copy
                            )
                        )

            def drain(n):
                for _ in range(min(n, len(fillers))):
                    fillers.popleft()[1]()

            def force_drain(key):
                while fillers and fillers[0][0] <= key:
                    fillers.popleft()[1]()

            def attn_block(qb, hp):
                force_drain((qb, hp))  # units this block's data depends on
                nkt = 4 * (qb + 1)
                oA = ps.tile([VW, 512], F32, tag="oA", bufs=1)
                oB = ps.tile([VW, 512], F32, tag="oB", bufs=1)
                # spread fillers over BOTH hp blocks of this qb
                budget = len(fillers)
                per_kt = -(-budget // ((2 - hp) * nkt)) if budget else 0
                for kt in range(nkt):
                    r = kt - 4 * qb
                    qlo = 128 * r if r > 0 else 0
                    s_t = ps.tile([128, 1024], F32, tag="s", bufs=2)
                    for h in (0, 1):
                        nc.tensor.matmul(
                            s_t[:, h * 512 + qlo : (h + 1) * 512],
                            kT_sb[h * 64 : (h + 1) * 64,
                                  hp * T + kt * 128 : hp * T + (kt + 1) * 128],
                            qT_sb[h * 64 : (h + 1) * 64,
                                  hp * T + qb * 512 + qlo : hp * T + (qb + 1) * 512],
                            start=True, stop=True, tile_position=(h * 64, 0),
                        )
                    e_t = ep.tile([128, 1024], BF16, tag="e")
                    if r > 0:
                        src = s_t[:].rearrange("p (h q) -> p h q", h=2)[:, :, qlo:]
                        dst = e_t[:].rearrange("p (h q) -> p h q", h=2)[:, :, qlo:]
                    else:
                        src = s_t[:]
                        dst = e_t[:]
                    nc.scalar.activation(
                        dst, src, mybir.ActivationFunctionType.Exp, scale=0.125
                    )
                    if r >= 0:  # diagonal: mask q < k + 128r
                        # (cols < 128r are skipped by the attv N-restriction,
                        # so only the triangle zone needs masking). DVE mask
                        # multiply, NOT gpsimd affine_select: mixing gpsimd op
                        # types forces Q7 library reloads that stall the
                        # partition_broadcast on the block tails.
                        er = e_t[:].rearrange("p (h q) -> p h q", h=2)
                        ape = er[:, :, 128 * r : 128 * (r + 1)]
                        nc.vector.tensor_mul(ape, ape, mskr)
                    drain(per_kt)
                    for h, oX in ((0, oA), (1, oB)):
                        nc.tensor.matmul(
                            oX[:, qlo:512],
                            vr[:, kt, 2 * hp + h, :],
                            e_t[:, h * 512 + qlo : (h + 1) * 512],
                            start=(kt == 0), stop=(kt == nkt - 1),
                        )
                # normalize -> attT. Stage oX into SBUF with one copy per head
                # (frees the PSUM bank for the next block immediately), then
                # bounce the denominator row to a base-0 tile: reciprocal_
                # approx_fast gives garbage on PSUM or non-base-0 operands.
                os_sb = nr.tile([VW, 1024], F32, tag="os")
                nc.vector.tensor_copy(os_sb[:, 0:512], oA[:])
                nc.vector.tensor_copy(os_sb[:, 512:1024], oB[:])
                dn = nr.tile([1, 1024], F32, tag="dn")
                nc.vector.tensor_copy(dn[:], os_sb[64:65, :])
                rr = nr.tile([1, 1024], F32, tag="rr")
                nc.vector.reciprocal_approx_fast(rr[:], dn[:])
                bc = nr.tile([64, 1024], F32, tag="bc")
                nc.gpsimd.partition_broadcast(bc[:], rr[:])
                cols = slice(hp * T + qb * 512, hp * T + (qb + 1) * 512)
                nc.vector.tensor_mul(
                    attT_sb[0:64, cols], os_sb[0:64, 0:512], bc[:, 0:512]
                )
                nc.vector.tensor_mul(
                    attT_sb[64:128, cols], os_sb[0:64, 512:1024], bc[:, 512:1024]
                )
                if DEBUG_DUMPS:
                    for nm, t in ((f"dos{qb}{hp}", dn), (f"drr{qb}{hp}", rr),
                                  (f"dbc{qb}{hp}", bc)):
                        dd = nc.dram_tensor(
                            nm, list(t.shape), F32, kind="ExternalOutput"
                        )
                        nc.sync.dma_start(dd.ap(), t[:])

            # ---- main schedule ----
            for _, u in proj_units(0):
                u()
            fillers.extend(proj_units(1))
            fillers.extend(proj_units(2))
            held = []
            for qb in range(NQB):
                if qb == 1:
                    fillers.extend(proj_units(3))
                if qb > 0:
                    units = list(oproj_units(qb - 1))
                    if qb == NQB - 1:  # reserve PE work for the final tail
                        held = units[5:]
                        units = units[:5]
                    fillers.extend(units)
                attn_block(qb, 0)
                attn_block(qb, 1)
            for _, u in held:
                u()
            drain(len(fillers))
            for _, u in oproj_units(NQB - 1, act_copy=True):
                u()

            if DEBUG_DUMPS:
                for nm, t in (
                    ("dq", qT_sb), ("dk", kT_sb), ("dv", v_sb), ("datt", attT_sb)
                ):
                    d = nc.dram_tensor(nm, list(t.shape), BF16, kind="ExternalOutput")
                    nc.sync.dma_start(d.ap(), t[:])

    nc.compile()
    return nc


def _prepack(w, dt):
    # [c*128, f] -> [128, c*f] (SBUF chunk layout)
    c = w.shape[0] // 128
    return np.ascontiguousarray(
        w.reshape(c, 128, w.shape[1]).transpose(1, 0, 2).reshape(128, -1)
    ).astype(dt)


def _prep_in_maps(x, Wq, Wk, Wv, Wo):
    x = np.asarray(x, dtype=np.float32)
    bf = mybir.dt.np(BF16)
    Wq = np.asarray(Wq, dtype=np.float32)
    Wk = np.asarray(Wk, dtype=np.float32)
    Wv = np.asarray(Wv, dtype=np.float32)
    Wo = np.asarray(Wo, dtype=np.float32)
    ii = np.arange(128)[:, None]
    jj = np.arange(128)[None, :]
    msk = np.tile((jj >= ii).astype(bf), (1, 2))
    in_maps = []
    for c in range(8):
        b, g = divmod(c, 4)
        sl = slice(g * FPC, (g + 1) * FPC)
        in_maps.append(
            {
                "xt": np.ascontiguousarray(x[b].T).astype(bf),
                "wq_t": _prepack(Wq[sl, :].T, bf),
                "wk_t": _prepack(Wk[sl, :].T, bf),
                "wv_t": _prepack(Wv[sl, :].T, bf),
                "wo_t": _prepack(Wo[:, sl].T, bf),
                "msk": msk,
            }
        )
    return in_maps


def _get_nc():
    if "nc" not in _CACHE:
        _CACHE["nc"] = _build()
    return _CACHE["nc"]


def _assemble(results):
    out = np.empty((B, T, D), dtype=np.float32)
    for b in range(B):
        out[b] = (
            results[4 * b]["po"].astype(np.float32)
            + results[4 * b + 1]["po"].astype(np.float32)
            + results[4 * b + 2]["po"].astype(np.float32)
            + results[4 * b + 3]["po"].astype(np.float32)
        )
    return out


def kernel(x, Wq, Wk, Wv, Wo):
    nc = _get_nc()
    in_maps = _prep_in_maps(x, Wq, Wk, Wv, Wo)
    res = run_bass_kernel_spmd(nc, in_maps, core_ids=list(range(8)))
    return _assemble(res.results)


def kernel_with_trace(x, Wq, Wk, Wv, Wo, **kw):
    nc = _get_nc()
    in_maps = _prep_in_maps(x, Wq, Wk, Wv, Wo)
    res = run_bass_kernel_spmd(nc, in_maps, core_ids=list(range(8)), trace=True, **kw)
    return _assemble(res.results), res
